# revision 7
# baseline (speedup 1.0000x reference)
"""Bass/Trainium2 kernel for nn_BigramLanguageModel (6-layer dense
transformer, B=128 T=256 C=384 H=6 V=65), data-parallel over batch on 8
NeuronCores.

Layout strategy (per core, 16 batches = 4096 tokens):
- residual stream x kept fp32 in natural [token, channel] layout
- LayerNorm gamma/beta folded into the weights host-side; on-device LN is
  just (x - mean) * rstd, applied by ScalarE with per-partition scale/bias
- activations transposed to [channel, token] with bf16 DMA-transposes so
  every matmul contracts over the partition dim
- attention softmax computed transposed ([s, q]); the denominator comes
  from an all-ones stationary-operand matmul which yields Z already
  broadcast across partitions; heads processed in pairs sharing the 128
  partitions; causal structure skips the dead quarter of the matmuls
- all matmuls bf16 with fp32 PSUM accumulation (embedding matmul in fp32)
"""
import numpy as np
import ml_dtypes

import concourse.bass as bass
import concourse.mybir as mybir
from concourse.bass_utils import run_bass_kernel_spmd
from concourse.tile import TileContext
from concourse.vector_clock import ScopedClock

F32 = mybir.dt.float32
BF16 = mybir.dt.bfloat16
AF = mybir.ActivationFunctionType
ALU = mybir.AluOpType
AX = mybir.AxisListType

V, C, BLOCK, H, L, D = 65, 384, 256, 6, 6, 64
B, T = 128, 256
F = 4 * C  # 1536
SCALE = C ** -0.5
NCORES = 8
B_SH = B // NCORES          # 16 batches per core
NTOK = B_SH * T             # 4096 tokens per core
NCH = C // 128              # 3 channel chunks
NFC = F // 128              # 12 ffn chunks
NPAIR = H // 2              # 3 head pairs
BF = ml_dtypes.bfloat16

# ---------------------------------------------------------------------------
# walrus in this container rejects >1 semaphore wait per instruction; peel
# extras onto same-engine nops (body commit hook + tail drain).
_MAXW = 1
_orig_add_instruction = TileContext._add_instruction


def _patched_add_instruction(self, inst):
    si = inst.sync_info
    if si is not None and si.on_wait and len(si.on_wait) > _MAXW:
        waits = list(si.on_wait)
        extra, keep = waits[:-_MAXW], waits[-_MAXW:]
        for i in range(0, len(extra), _MAXW):
            nop = mybir.InstNoOp(
                name=self.nc.get_next_instruction_name(),
                engine=inst.engine,
                sync_info=mybir.SyncInfo(on_wait=extra[i : i + _MAXW], on_update=[]),
                bass_nofuse=True,
            )
            _orig_add_instruction(self, nop)
        inst.sync_info = mybir.SyncInfo(on_wait=keep, on_update=list(si.on_update or []))
    _orig_add_instruction(self, inst)


def _patched_drain_and_barrier(self, tick_clock, wait_clock):
    carrier = self.nc.sync.nop()
    wait_clock.add_sem_waits(carrier.ins, ScopedClock({None: tick_clock.global_clock}))
    si = carrier.ins.sync_info
    waits = list(si.on_wait) if si is not None and si.on_wait else []
    if len(waits) > _MAXW:
        si.on_wait = waits[:_MAXW]
        for i in range(_MAXW, len(waits), _MAXW):
            n = self.nc.sync.nop()
            n.ins.sync_info = mybir.SyncInfo(on_wait=waits[i : i + _MAXW], on_update=[])
    self.nc.sync.drain()
    self.nc.all_engine_barrier()
    popped = self.nc._tile_sem_poison_stack.pop()
    assert popped is self._sem_poison
    self.nc.clear_and_free_semaphores(list(self.sems.allocated().values()))
    self.nc.all_engine_barrier()


TileContext._add_instruction = _patched_add_instruction
TileContext._drain_and_barrier = _patched_drain_and_barrier
# ---------------------------------------------------------------------------


def _np(x, dt=np.float32):
    return np.ascontiguousarray(np.asarray(x, dtype=dt))


def prep_params(params):
    """Fold LN gamma/beta into weights; pack per-layer SBUF images."""
    out = {}
    wqk = np.zeros((L, 128, H * 3 * 128), BF)
    bqk = np.zeros((L, 128, H), np.float32)
    wv = np.zeros((L, 128, NPAIR * 3 * 128), BF)
    wp = np.zeros((L, 128, 3 * C), BF)
    bpb2 = np.zeros((L, 2, C), BF)
    w1 = np.zeros((L, 128, 3 * NFC * 128), BF)
    b1c = np.zeros((L, 128, NFC), np.float32)
    w2 = np.zeros((L, 128, NFC * C), BF)
    for l, p in enumerate(params["blocks"]):
        g1, be1 = _np(p["ln1_g"]), _np(p["ln1_b"])
        g2, be2 = _np(p["ln2_g"]), _np(p["ln2_b"])
        Wq, Wk, Wv_ = _np(p["Wq"]), _np(p["Wk"]), _np(p["Wv"])  # [H, C, D]
        Wp, bp = _np(p["Wp"]), _np(p["bp"])
        W1, b1 = _np(p["W1"]), _np(p["b1"])
        W2, b2 = _np(p["W2"]), _np(p["b2"])
        gWq = g1[None, :, None] * Wq
        gWk = g1[None, :, None] * Wk
        gWv = g1[None, :, None] * Wv_
        bWq = np.einsum("c,hcd->hd", be1, Wq)
        bWk = np.einsum("c,hcd->hd", be1, Wk)
        bWv = np.einsum("c,hcd->hd", be1, Wv_)  # folded into proj bias
        for h in range(H):
            for ch in range(NCH):
                blk = np.concatenate(
                    [gWq[h, ch * 128 : (ch + 1) * 128, :],
                     gWk[h, ch * 128 : (ch + 1) * 128, :]], axis=1)
                wqk[l, :, (h * 3 + ch) * 128 : (h * 3 + ch + 1) * 128] = blk.astype(BF)
            bqk[l, 0:64, h] = bWq[h]
            bqk[l, 64:128, h] = bWk[h]
        for pr in range(NPAIR):
            for ch in range(NCH):
                blk = np.concatenate(
                    [gWv[2 * pr, ch * 128 : (ch + 1) * 128, :],
                     gWv[2 * pr + 1, ch * 128 : (ch + 1) * 128, :]], axis=1)
                wv[l, :, (pr * 3 + ch) * 128 : (pr * 3 + ch + 1) * 128] = blk.astype(BF)
        for ch in range(NCH):
            wp[l, :, ch * C : (ch + 1) * C] = Wp[ch * 128 : (ch + 1) * 128, :].astype(BF)
        bp_adj = bp + bWv.reshape(C) @ Wp  # softmax rows sum to 1
        bpb2[l, 0, :] = bp_adj.astype(BF)
        bpb2[l, 1, :] = b2.astype(BF)
        gW1 = g2[:, None] * W1
        bW1 = be2 @ W1
        for ch in range(NCH):
            for fc in range(NFC):
                w1[l, :, (ch * NFC + fc) * 128 : (ch * NFC + fc + 1) * 128] = gW1[
                    ch * 128 : (ch + 1) * 128, fc * 128 : (fc + 1) * 128].astype(BF)
        for fc in range(NFC):
            b1c[l, :, fc] = (b1 + bW1)[fc * 128 : (fc + 1) * 128]
            w2[l, :, fc * C : (fc + 1) * C] = W2[fc * 128 : (fc + 1) * 128, :].astype(BF)
    out.update(wqk=wqk, bqk=bqk, wv=wv, wp=wp, bpb2=bpb2, w1=w1, b1c=b1c, w2=w2)

    fW1, fb1 = _np(params["fW1"]), _np(params["fb1"])
    fW2, fb2 = _np(params["fW2"]), _np(params["fb2"])
    fw1 = np.zeros((128, 3 * NFC * 128), BF)
    fb1c = np.zeros((128, NFC), np.float32)
    fw2 = np.zeros((128, NFC * C), BF)
    for ch in range(NCH):
        for fc in range(NFC):
            fw1[:, (ch * NFC + fc) * 128 : (ch * NFC + fc + 1) * 128] = fW1[
                ch * 128 : (ch + 1) * 128, fc * 128 : (fc + 1) * 128].astype(BF)
    for fc in range(NFC):
        fb1c[:, fc] = fb1[fc * 128 : (fc + 1) * 128]
        fw2[:, fc * C : (fc + 1) * C] = fW2[fc * 128 : (fc + 1) * 128, :].astype(BF)
    out.update(fw1=fw1, fb1c=fb1c, fw2=fw2, fb2=fb2.reshape(1, C).astype(BF))

    lm_W, lm_b = _np(params["lm_W"]), _np(params["lm_b"])
    lmw = np.zeros((128, NCH * V), BF)
    for ch in range(NCH):
        lmw[:, ch * V : (ch + 1) * V] = lm_W[ch * 128 : (ch + 1) * 128, :].astype(BF)
    out["lmw"] = lmw
    out["lmb"] = lm_b.reshape(V, 1).astype(np.float32)

    out["tok"] = _np(params["tok"])  # [65, 384] f32
    out["pos"] = _np(params["pos"]).reshape(2, 128, C).transpose(1, 0, 2).reshape(128, 2 * C)
    out["iota"] = np.tile(np.arange(V, dtype=np.float32), (128, 1))
    out["ident"] = np.eye(128, dtype=np.float32)
    out["cmask"] = (np.arange(256)[None, :] >= np.arange(128)[:, None]).astype(BF)
    return out


def build(nc, L_RUN=L, NB=B_SH, debug=()):
    """Emit the model for NB batch units and L_RUN layers.

    debug: tap names among {"emb", "h1", "xattn0", "xlayer0", ...} added as
    extra outputs.
    """
    NTT_r = NB * 2                 # 128-token tiles
    ntok_r = NB * 256
    TBS = min(512, ntok_r)         # ffn/lm token block size
    NTB_r = ntok_r // TBS

    d = {}
    def din(name, shape, dt):
        d[name] = nc.dram_tensor(name, shape, dt, kind="ExternalInput")
    din("wqk", [L, 128, H * 3 * 128], BF16)
    din("bqk", [L, 128, H], F32)
    din("wv", [L, 128, NPAIR * 3 * 128], BF16)
    din("wp", [L, 128, 3 * C], BF16)
    din("bpb2", [L, 2, C], BF16)
    din("w1", [L, 128, 3 * NFC * 128], BF16)
    din("b1c", [L, 128, NFC], F32)
    din("w2", [L, 128, NFC * C], BF16)
    din("fw1", [128, 3 * NFC * 128], BF16)
    din("fb1c", [128, NFC], F32)
    din("fw2", [128, NFC * C], BF16)
    din("fb2", [1, C], BF16)
    din("lmw", [128, NCH * V], BF16)
    din("lmb", [V, 1], F32)
    din("tok", [V, C], F32)
    din("pos", [128, 2 * C], F32)
    din("iota", [128, V], F32)
    din("ident", [128, 128], F32)
    din("cmask", [128, 256], BF16)
    din("idxf", [ntok_r, 1], F32)
    din("tgtf", [ntok_r, 1], F32)

    logits_out = nc.dram_tensor("logits", [ntok_r, V], F32, kind="ExternalOutput")
    loss_out = nc.dram_tensor("loss_sum", [1, 1], F32, kind="ExternalOutput")
    taps = {}
    for name in debug:
        dt = BF16 if name in ("h1",) else F32
        taps[name] = nc.dram_tensor("tap_" + name, [NTT_r * 128, C], dt,
                                    kind="ExternalOutput")

    with TileContext(nc) as tc:
        with (
            tc.tile_pool(name="const", bufs=1) as cpool,
            tc.tile_pool(name="wts", bufs=2) as wpool,
            tc.tile_pool(name="big", bufs=1) as bigpool,
            tc.tile_pool(name="work", bufs=3) as wk,
            tc.tile_pool(name="h1p", bufs=2) as h1p,
            tc.tile_pool(name="ps", bufs=2, space="PSUM") as psA,
            tc.tile_pool(name="psoz", bufs=4, space="PSUM") as psOZ,
            tc.tile_pool(name="psv", bufs=2, space="PSUM") as psV,
        ):
            # ---- constants -------------------------------------------------
            tok_sb = cpool.tile([V, C], F32, tag="tok")
            nc.sync.dma_start(tok_sb[:], d["tok"][:])
            pos_sb = cpool.tile([128, 2 * C], F32, tag="pos")
            nc.sync.dma_start(pos_sb[:], d["pos"][:])
            iota_sb = cpool.tile([128, V], F32, tag="iota")
            nc.sync.dma_start(iota_sb[:], d["iota"][:])
            ident_sb = cpool.tile([128, 128], F32, tag="ident")
            nc.sync.dma_start(ident_sb[:], d["ident"][:])
            cmask_sb = cpool.tile([128, 256], BF16, tag="cmask")
            nc.sync.dma_start(cmask_sb[:], d["cmask"][:])
            ones64 = cpool.tile([128, 64], BF16, tag="ones64")
            nc.vector.memset(ones64[:], 1.0)
            ones1 = cpool.tile([1, 128], BF16, tag="ones1")
            nc.vector.memset(ones1[:], 1.0)
            lmw_sb = cpool.tile([128, NCH * V], BF16, tag="lmw")
            nc.sync.dma_start(lmw_sb[:], d["lmw"][:])
            lmb_sb = cpool.tile([V, 1], F32, tag="lmb")
            nc.sync.dma_start(lmb_sb[:], d["lmb"][:])
            loss_acc = cpool.tile([128, 1], F32, tag="lacc")
            nc.vector.memset(loss_acc[:], 0.0)
            eps_sb = cpool.tile([128, 1], F32, tag="eps")
            nc.vector.memset(eps_sb[:], 1e-5)

            # persistent residual stream
            xs = [bigpool.tile([128, C], F32, tag=f"x{t}", name=f"x{t}") for t in range(NTT_r)]

            def dump_x(name):
                if name in taps:
                    for tt in range(NTT_r):
                        nc.sync.dma_start(
                            taps[name][tt * 128 : (tt + 1) * 128, :], xs[tt][:])

            # ---- embedding -------------------------------------------------
            for tt in range(NTT_r):
                idxc = wk.tile([128, 1], F32, tag="idxc")
                nc.sync.dma_start(idxc[:], d["idxf"][tt * 128 : (tt + 1) * 128, :])
                oh = wk.tile([128, V], F32, tag="oh")
                nc.vector.tensor_scalar(oh[:], iota_sb[:], idxc[:], None, ALU.is_equal)
                ohT = psA.tile([V, 128], F32, tag="mmA")
                nc.tensor.transpose(ohT[:], oh[:], ident_sb[:])
                ohT_sb = wk.tile([V, 128], F32, tag="ohT")
                nc.scalar.copy(ohT_sb[:], ohT[:])
                x0 = psA.tile([128, C], F32, tag="mmA")
                nc.tensor.matmul(x0[:], ohT_sb[:], tok_sb[:], start=True, stop=True)
                nc.vector.tensor_tensor(
                    xs[tt][:], x0[:], pos_sb[:, (tt % 2) * C : (tt % 2 + 1) * C], ALU.add)
            dump_x("emb")

            def layer_norm_T(apply_ln=True, tapname=None):
                """LN(x) (or plain bf16 cast) -> DMA-transpose into 3
                [128, ntok_r] chunk tiles."""
                hT = [bigpool.tile([128, ntok_r], BF16, tag=f"hT{j}", name=f"hT{j}") for j in range(NCH)]
                for tt in range(NTT_r):
                    hb = wk.tile([128, C], BF16, tag="hb")
                    if apply_ln:
                        st6 = wk.tile([128, 6], F32, tag="st6")
                        nc.vector.bn_stats(st6[:], xs[tt][:])
                        mv = wk.tile([128, 2], F32, tag="mv")
                        nc.vector.bn_aggr(mv[:], st6[:])
                        sd = wk.tile([128, 1], F32, tag="sd")
                        nc.scalar.activation(sd[:], mv[:, 1:2], AF.Sqrt, bias=eps_sb[:])
                        rstd = wk.tile([128, 1], F32, tag="rstd")
                        nc.vector.reciprocal(rstd[:], sd[:])
                        nmr = wk.tile([128, 1], F32, tag="nmr")
                        nc.vector.scalar_tensor_tensor(
                            nmr[:], mv[:, 0:1], -1.0, rstd[:], ALU.mult, ALU.mult)
                        nc.scalar.activation(
                            hb[:], xs[tt][:], AF.Identity, bias=nmr[:], scale=rstd[:])
                    else:
                        nc.scalar.copy(hb[:], xs[tt][:])
                    if tapname is not None and tapname in taps:
                        nc.sync.dma_start(
                            taps[tapname][tt * 128 : (tt + 1) * 128, :], hb[:])
                    for j in range(NCH):
                        nc.sync.dma_start_transpose(
                            hT[j][:, tt * 128 : (tt + 1) * 128],
                            hb[:, j * 128 : (j + 1) * 128])
                return hT

            # ---- transformer layers ---------------------------------------
            for l in range(L_RUN):
                wqk_sb = wpool.tile([128, H * 3 * 128], BF16, tag="wqk")
                nc.sync.dma_start(wqk_sb[:], d["wqk"][l])
                bqk_sb = wpool.tile([128, H], F32, tag="bqk")
                nc.sync.dma_start(bqk_sb[:], d["bqk"][l])
                wv_sb = wpool.tile([128, NPAIR * 3 * 128], BF16, tag="wv")
                nc.sync.dma_start(wv_sb[:], d["wv"][l])
                wp_sb = wpool.tile([128, 3 * C], BF16, tag="wp")
                nc.sync.dma_start(wp_sb[:], d["wp"][l])
                bp_sb = wpool.tile([1, C], BF16, tag="bprow")
                nc.sync.dma_start(bp_sb[:], d["bpb2"][l, 0:1, :])
                b2_sb = wpool.tile([1, C], BF16, tag="b2row")
                nc.sync.dma_start(b2_sb[:], d["bpb2"][l, 1:2, :])
                w1_sb = wpool.tile([128, 3 * NFC * 128], BF16, tag="w1")
                nc.sync.dma_start(w1_sb[:], d["w1"][l])
                b1_sb = wpool.tile([128, NFC], F32, tag="b1c")
                nc.sync.dma_start(b1_sb[:], d["b1c"][l])
                w2_sb = wpool.tile([128, NFC * C], BF16, tag="w2")
                nc.sync.dma_start(w2_sb[:], d["w2"][l])

                hT = layer_norm_T(apply_ln=True, tapname="h1" if l == 0 else None)

                for b in range(NB):
                    t0 = b * 256
                    v_sb = []
                    for pr in range(NPAIR):
                        vp = []
                        for tch in range(2):
                            vps = psV.tile([128, 128], F32, tag="v")
                            for ch in range(NCH):
                                nc.tensor.matmul(
                                    vps[:],
                                    hT[ch][:, t0 + tch * 128 : t0 + (tch + 1) * 128],
                                    wv_sb[:, (pr * 3 + ch) * 128 : (pr * 3 + ch + 1) * 128],
                                    start=(ch == 0), stop=(ch == NCH - 1))
                            vsb = wk.tile([128, 128], BF16, tag=f"vsb{pr}_{tch}")
                            nc.scalar.copy(vsb[:], vps[:])
                            vp.append(vsb)
                        v_sb.append(vp)

                    ostks = []
                    for pr in range(NPAIR):
                        e_t = []
                        for hh in range(2):
                            h = 2 * pr + hh
                            qk = psA.tile([128, 256], F32, tag="mmA")
                            for ch in range(NCH):
                                nc.tensor.matmul(
                                    qk[:],
                                    wqk_sb[:, (h * 3 + ch) * 128 : (h * 3 + ch + 1) * 128],
                                    hT[ch][:, t0 : t0 + 256],
                                    start=(ch == 0), stop=(ch == NCH - 1))
                            qksb = wk.tile([128, 256], BF16, tag="qksb")
                            nc.scalar.activation(
                                qksb[:], qk[:], AF.Identity, bias=bqk_sb[:, h : h + 1])
                            ksp = wk.tile([64, 256], BF16, tag="ksp")
                            nc.sync.dma_start(ksp[:], qksb[64:128, :])
                            sc = psA.tile([128, 384], F32, tag="mmA")
                            nc.tensor.matmul(
                                sc[:, 0:256], ksp[:, 0:128], qksb[0:64, :],
                                start=True, stop=True)
                            nc.tensor.matmul(
                                sc[:, 256:384], ksp[:, 128:256], qksb[0:64, 128:256],
                                start=True, stop=True)
                            e = wk.tile([128, 384], BF16, tag="e")
                            nc.scalar.activation(e[:, 0:256], sc[:, 0:256], AF.Exp, scale=SCALE)
                            nc.scalar.activation(e[:, 256:384], sc[:, 256:384], AF.Exp, scale=SCALE)
                            nc.vector.tensor_tensor(e[:, 0:256], e[:, 0:256], cmask_sb[:], ALU.mult)
                            nc.vector.tensor_tensor(
                                e[:, 256:384], e[:, 256:384], cmask_sb[:, 0:128], ALU.mult)
                            e_t.append(e)

                        opair = psOZ.tile([128, 256], F32, tag="oz")
                        zpair = psOZ.tile([128, 256], F32, tag="oz")
                        for hh in range(2):
                            e = e_t[hh]
                            r0, r1 = hh * 64, (hh + 1) * 64
                            nc.tensor.matmul(
                                opair[r0:r1, 0:256], v_sb[pr][0][:, hh * 64 : (hh + 1) * 64],
                                e[:, 0:256], start=True, stop=False)
                            nc.tensor.matmul(
                                opair[r0:r1, 128:256], v_sb[pr][1][:, hh * 64 : (hh + 1) * 64],
                                e[:, 256:384], start=False, stop=True)
                            nc.tensor.matmul(
                                zpair[r0:r1, 0:256], ones64[:, 0:64],
                                e[:, 0:256], start=True, stop=False)
                            nc.tensor.matmul(
                                zpair[r0:r1, 128:256], ones64[:, 0:64],
                                e[:, 256:384], start=False, stop=True)
                        rz = wk.tile([128, 256], BF16, tag="rz")
                        with nc.allow_low_precision(reason="softmax denom bf16"):
                            nc.vector.reciprocal(rz[:], zpair[:])
                        osb = wk.tile([128, 256], BF16, tag="osb")
                        nc.scalar.copy(osb[:], opair[:])
                        ostk = wk.tile([128, 256], BF16, tag=f"ostk{pr}")
                        nc.vector.tensor_tensor(ostk[:], osb[:], rz[:], ALU.mult)
                        ostks.append(ostk)

                    for th in range(2):
                        tt = 2 * b + th
                        dx = psA.tile([128, C], F32, tag="mmA")
                        for pr in range(NPAIR):
                            nc.tensor.matmul(
                                dx[:],
                                ostks[pr][:, th * 128 : (th + 1) * 128],
                                wp_sb[:, pr * C : (pr + 1) * C],
                                start=(pr == 0), stop=False)
                        nc.tensor.matmul(dx[:], ones1[:], bp_sb[0:1, :],
                                         start=False, stop=True)
                        nc.vector.tensor_tensor(xs[tt][:], xs[tt][:], dx[:], ALU.add)
                dump_x(f"xattn{l}")

                h2T = layer_norm_T(apply_ln=True)
                for tb in range(NTB_r):
                    h1s = []
                    for fc in range(NFC):
                        h1 = psA.tile([128, TBS], F32, tag="mmA")
                        for ch in range(NCH):
                            nc.tensor.matmul(
                                h1[:],
                                w1_sb[:, (ch * NFC + fc) * 128 : (ch * NFC + fc + 1) * 128],
                                h2T[ch][:, tb * TBS : (tb + 1) * TBS],
                                start=(ch == 0), stop=(ch == NCH - 1))
                        h1sb = h1p.tile([128, TBS], BF16, tag=f"h1_{fc}")
                        nc.scalar.activation(h1sb[:], h1[:], AF.Relu,
                                             bias=b1_sb[:, fc : fc + 1])
                        h1s.append(h1sb)
                    for t4 in range(TBS // 128):
                        tt = tb * (TBS // 128) + t4
                        dx = psA.tile([128, C], F32, tag="mmA")
                        for fc in range(NFC):
                            nc.tensor.matmul(
                                dx[:],
                                h1s[fc][:, t4 * 128 : (t4 + 1) * 128],
                                w2_sb[:, fc * C : (fc + 1) * C],
                                start=(fc == 0), stop=False)
                        nc.tensor.matmul(dx[:], ones1[:], b2_sb[0:1, :],
                                         start=False, stop=True)
                        nc.vector.tensor_tensor(xs[tt][:], xs[tt][:], dx[:], ALU.add)
                dump_x(f"xlayer{l}")

            # ---- final FFN (no LN, no residual) + LM head + loss ----------
            fw1_sb = wpool.tile([128, 3 * NFC * 128], BF16, tag="w1")
            nc.sync.dma_start(fw1_sb[:], d["fw1"][:])
            fb1_sb = wpool.tile([128, NFC], F32, tag="b1c")
            nc.sync.dma_start(fb1_sb[:], d["fb1c"][:])
            fw2_sb = wpool.tile([128, NFC * C], BF16, tag="w2")
            nc.sync.dma_start(fw2_sb[:], d["fw2"][:])
            fb2_sb = wpool.tile([1, C], BF16, tag="b2row")
            nc.sync.dma_start(fb2_sb[:], d["fb2"][:])

            xT = layer_norm_T(apply_ln=False)
            for tb in range(NTB_r):
                h1s = []
                for fc in range(NFC):
                    h1 = psA.tile([128, TBS], F32, tag="mmA")
                    for ch in range(NCH):
                        nc.tensor.matmul(
                            h1[:],
                            fw1_sb[:, (ch * NFC + fc) * 128 : (ch * NFC + fc + 1) * 128],
                            xT[ch][:, tb * TBS : (tb + 1) * TBS],
                            start=(ch == 0), stop=(ch == NCH - 1))
                    h1sb = h1p.tile([128, TBS], BF16, tag=f"h1_{fc}")
                    nc.scalar.activation(h1sb[:], h1[:], AF.Relu,
                                         bias=fb1_sb[:, fc : fc + 1])
                    h1s.append(h1sb)
                yTb = [wk.tile([128, TBS], BF16, tag=f"yTb{j}", name=f"yTb{j}") for j in range(NCH)]
                for t4 in range(TBS // 128):
                    tt = tb * (TBS // 128) + t4
                    dx = psA.tile([128, C], F32, tag="mmA")
                    for fc in range(NFC):
                        nc.tensor.matmul(
                            dx[:],
                            h1s[fc][:, t4 * 128 : (t4 + 1) * 128],
                            fw2_sb[:, fc * C : (fc + 1) * C],
                            start=(fc == 0), stop=False)
                    nc.tensor.matmul(dx[:], ones1[:], fb2_sb[0:1, :],
                                     start=False, stop=True)
                    yb = wk.tile([128, C], BF16, tag="hb")
                    nc.scalar.copy(yb[:], dx[:])
                    for j in range(NCH):
                        nc.sync.dma_start_transpose(
                            yTb[j][:, t4 * 128 : (t4 + 1) * 128],
                            yb[:, j * 128 : (j + 1) * 128])

                lgT = psA.tile([V, TBS], F32, tag="mmA")
                for ch in range(NCH):
                    nc.tensor.matmul(
                        lgT[:], lmw_sb[:, ch * V : (ch + 1) * V], yTb[ch][:],
                        start=(ch == 0), stop=(ch == NCH - 1))
                lgT_sb = wk.tile([V, TBS], F32, tag="lgT")
                nc.scalar.activation(lgT_sb[:], lgT[:], AF.Identity, bias=lmb_sb[:, 0:1])
                for t4 in range(TBS // 128):
                    tt = tb * (TBS // 128) + t4
                    lgn = psA.tile([128, V], F32, tag="mmA")
                    nc.tensor.transpose(
                        lgn[:], lgT_sb[:, t4 * 128 : (t4 + 1) * 128],
                        ident_sb[0:V, 0:V])
                    lg = wk.tile([128, V], F32, tag="lg")
                    nc.scalar.copy(lg[:], lgn[:])
                    nc.sync.dma_start(logits_out[tt * 128 : (tt + 1) * 128, :], lg[:])
                    tgtc = wk.tile([128, 1], F32, tag="tgtc")
                    nc.sync.dma_start(tgtc[:], d["tgtf"][tt * 128 : (tt + 1) * 128, :])
                    ohg = wk.tile([128, V], F32, tag="ohg")
                    nc.vector.tensor_scalar(ohg[:], iota_sb[:], tgtc[:], None, ALU.is_equal)
                    mx = wk.tile([128, 1], F32, tag="mx")
                    nc.vector.tensor_reduce(mx[:], lg[:], AX.X, ALU.max)
                    nmx = wk.tile([128, 1], F32, tag="nmx")
                    nc.vector.tensor_scalar_mul(nmx[:], mx[:], -1.0)
                    esc = wk.tile([128, V], F32, tag="esc")
                    zc = wk.tile([128, 1], F32, tag="zc")
                    nc.scalar.activation(esc[:], lg[:], AF.Exp, bias=nmx[:], accum_out=zc[:])
                    lnz = wk.tile([128, 1], F32, tag="lnz")
                    nc.scalar.activation(lnz[:], zc[:], AF.Ln)
                    tscr = wk.tile([128, V], F32, tag="tscr")
                    tgl = wk.tile([128, 1], F32, tag="tgl")
                    nc.vector.scalar_tensor_tensor(
                        tscr[:], lg[:], 1.0, ohg[:], ALU.mult, ALU.mult, accum_out=tgl[:])
                    lt = wk.tile([128, 1], F32, tag="lt")
                    nc.vector.scalar_tensor_tensor(
                        lt[:], lnz[:], mx[:], tgl[:], ALU.add, ALU.subtract)
                    nc.vector.tensor_tensor(loss_acc[:], loss_acc[:], lt[:], ALU.add)
            lsum = cpool.tile([1, 1], F32, tag="lsum")
            nc.gpsimd.tensor_reduce(lsum[:], loss_acc[:], AX.C, ALU.add)
            nc.sync.dma_start(loss_out[:], lsum[:])
    return d


def make_in_map(pp, idx_sh, tgt_sh):
    m = dict(pp)
    ntok = idx_sh.size
    m["idxf"] = idx_sh.reshape(ntok, 1).astype(np.float32)
    m["tgtf"] = tgt_sh.reshape(ntok, 1).astype(np.float32)
    return m


def kernel(params=None, idx=None, targets=None):
    pp = prep_params(params)
    idx = _np(idx, np.int64)
    targets = _np(targets, np.int64)

    nc = bass.Bass()
    build(nc)

    in_maps = [
        make_in_map(pp, idx[c * B_SH : (c + 1) * B_SH], targets[c * B_SH : (c + 1) * B_SH])
        for c in range(NCORES)
    ]
    res = run_bass_kernel_spmd(nc, in_maps, list(range(NCORES))).results

    logits = np.concatenate(
        [r["logits"].reshape(B_SH, T, V) for r in res], axis=0).astype(np.float32)
    loss = np.float32(sum(float(r["loss_sum"][0, 0]) for r in res) / (B * T))
    return logits, loss


# revision 17
# speedup vs baseline: 1.5783x; 1.5783x over previous
"""Bass/Trainium2 kernel for nn_BigramLanguageModel (6-layer dense
transformer, B=128 T=256 C=384 H=6 V=65), data-parallel over batch on 8
NeuronCores.

Layout strategy (per core, 16 batches = 4096 tokens):
- residual stream x kept fp32 in natural [token, channel] layout
- LayerNorm gamma/beta folded into the weights host-side; on-device LN is
  just (x - mean) * rstd, applied by ScalarE with per-partition scale/bias
- activations transposed to [channel, token] with bf16 DMA-transposes so
  every matmul contracts over the partition dim
- attention softmax computed transposed ([s, q]); the denominator comes
  from an all-ones stationary-operand matmul which yields Z already
  broadcast across partitions; heads processed in pairs sharing the 128
  partitions; causal structure skips the dead quarter of the matmuls
- all matmuls bf16 with fp32 PSUM accumulation (embedding matmul in fp32)
"""
import numpy as np
import ml_dtypes

import concourse.bass as bass
import concourse.mybir as mybir
from concourse.bass_utils import run_bass_kernel_spmd
from concourse.tile import TileContext
from concourse.vector_clock import ScopedClock

F32 = mybir.dt.float32
BF16 = mybir.dt.bfloat16
AF = mybir.ActivationFunctionType
ALU = mybir.AluOpType
AX = mybir.AxisListType

V, C, BLOCK, H, L, D = 65, 384, 256, 6, 6, 64
B, T = 128, 256
F = 4 * C  # 1536
SCALE = C ** -0.5
NCORES = 8
B_SH = B // NCORES          # 16 batches per core
NTOK = B_SH * T             # 4096 tokens per core
NCH = C // 128              # 3 channel chunks
NFC = F // 128              # 12 ffn chunks
NPAIR = H // 2              # 3 head pairs
BF = ml_dtypes.bfloat16

# ---------------------------------------------------------------------------
# walrus in this container rejects >1 semaphore wait per instruction; peel
# extras onto same-engine nops (body commit hook + tail drain).
_MAXW = 1
_orig_add_instruction = TileContext._add_instruction


def _patched_add_instruction(self, inst):
    si = inst.sync_info
    if si is not None and si.on_wait and len(si.on_wait) > _MAXW:
        waits = list(si.on_wait)
        extra, keep = waits[:-_MAXW], waits[-_MAXW:]
        for i in range(0, len(extra), _MAXW):
            nop = mybir.InstNoOp(
                name=self.nc.get_next_instruction_name(),
                engine=inst.engine,
                sync_info=mybir.SyncInfo(on_wait=extra[i : i + _MAXW], on_update=[]),
                bass_nofuse=True,
            )
            _orig_add_instruction(self, nop)
        inst.sync_info = mybir.SyncInfo(on_wait=keep, on_update=list(si.on_update or []))
    _orig_add_instruction(self, inst)


def _patched_drain_and_barrier(self, tick_clock, wait_clock):
    carrier = self.nc.sync.nop()
    wait_clock.add_sem_waits(carrier.ins, ScopedClock({None: tick_clock.global_clock}))
    si = carrier.ins.sync_info
    waits = list(si.on_wait) if si is not None and si.on_wait else []
    if len(waits) > _MAXW:
        si.on_wait = waits[:_MAXW]
        for i in range(_MAXW, len(waits), _MAXW):
            n = self.nc.sync.nop()
            n.ins.sync_info = mybir.SyncInfo(on_wait=waits[i : i + _MAXW], on_update=[])
    self.nc.sync.drain()
    self.nc.all_engine_barrier()
    popped = self.nc._tile_sem_poison_stack.pop()
    assert popped is self._sem_poison
    self.nc.clear_and_free_semaphores(list(self.sems.allocated().values()))
    self.nc.all_engine_barrier()


TileContext._add_instruction = _patched_add_instruction
TileContext._drain_and_barrier = _patched_drain_and_barrier
# ---------------------------------------------------------------------------


def _np(x, dt=np.float32):
    return np.ascontiguousarray(np.asarray(x, dtype=dt))


def prep_params(params):
    """Fold LN gamma/beta into weights; pack per-layer SBUF images."""
    out = {}
    wqk = np.zeros((L, 128, H * 3 * 128), BF)
    bqk = np.zeros((L, 128, H), np.float32)
    wv = np.zeros((L, 128, NPAIR * 3 * 128), BF)
    wp = np.zeros((L, 128, 3 * C), BF)
    bpb2 = np.zeros((L, 2, C), BF)
    w1 = np.zeros((L, 128, 3 * NFC * 128), BF)
    b1c = np.zeros((L, 128, NFC), np.float32)
    w2 = np.zeros((L, 128, NFC * C), BF)
    for l, p in enumerate(params["blocks"]):
        g1, be1 = _np(p["ln1_g"]), _np(p["ln1_b"])
        g2, be2 = _np(p["ln2_g"]), _np(p["ln2_b"])
        Wq, Wk, Wv_ = _np(p["Wq"]), _np(p["Wk"]), _np(p["Wv"])  # [H, C, D]
        Wp, bp = _np(p["Wp"]), _np(p["bp"])
        W1, b1 = _np(p["W1"]), _np(p["b1"])
        W2, b2 = _np(p["W2"]), _np(p["b2"])
        gWq = g1[None, :, None] * Wq
        gWk = g1[None, :, None] * Wk
        gWv = g1[None, :, None] * Wv_
        bWq = np.einsum("c,hcd->hd", be1, Wq)
        bWk = np.einsum("c,hcd->hd", be1, Wk)
        bWv = np.einsum("c,hcd->hd", be1, Wv_)  # folded into proj bias
        for h in range(H):
            for ch in range(NCH):
                blk = np.concatenate(
                    [gWq[h, ch * 128 : (ch + 1) * 128, :],
                     gWk[h, ch * 128 : (ch + 1) * 128, :]], axis=1)
                wqk[l, :, (h * 3 + ch) * 128 : (h * 3 + ch + 1) * 128] = blk.astype(BF)
            bqk[l, 0:64, h] = bWq[h]
            bqk[l, 64:128, h] = bWk[h]
        for pr in range(NPAIR):
            for ch in range(NCH):
                blk = np.concatenate(
                    [gWv[2 * pr, ch * 128 : (ch + 1) * 128, :],
                     gWv[2 * pr + 1, ch * 128 : (ch + 1) * 128, :]], axis=1)
                wv[l, :, (ch * NPAIR + pr) * 128 : (ch * NPAIR + pr + 1) * 128] = blk.astype(BF)
        for ch in range(NCH):
            wp[l, :, ch * C : (ch + 1) * C] = Wp[ch * 128 : (ch + 1) * 128, :].astype(BF)
        bp_adj = bp + bWv.reshape(C) @ Wp  # softmax rows sum to 1
        bpb2[l, 0, :] = bp_adj.astype(BF)
        bpb2[l, 1, :] = b2.astype(BF)
        gW1 = g2[:, None] * W1
        bW1 = be2 @ W1
        for ch in range(NCH):
            for fc in range(NFC):
                w1[l, :, (ch * NFC + fc) * 128 : (ch * NFC + fc + 1) * 128] = gW1[
                    ch * 128 : (ch + 1) * 128, fc * 128 : (fc + 1) * 128].astype(BF)
        for fc in range(NFC):
            b1c[l, :, fc] = (b1 + bW1)[fc * 128 : (fc + 1) * 128]
            w2[l, :, fc * C : (fc + 1) * C] = W2[fc * 128 : (fc + 1) * 128, :].astype(BF)
    out.update(wqk=wqk, bqk=bqk, wv=wv, wp=wp, bpb2=bpb2, w1=w1, b1c=b1c, w2=w2)

    fW1, fb1 = _np(params["fW1"]), _np(params["fb1"])
    fW2, fb2 = _np(params["fW2"]), _np(params["fb2"])
    fw1 = np.zeros((128, 3 * NFC * 128), BF)
    fb1c = np.zeros((128, NFC), np.float32)
    fw2 = np.zeros((128, NFC * C), BF)
    for ch in range(NCH):
        for fc in range(NFC):
            fw1[:, (ch * NFC + fc) * 128 : (ch * NFC + fc + 1) * 128] = fW1[
                ch * 128 : (ch + 1) * 128, fc * 128 : (fc + 1) * 128].astype(BF)
    for fc in range(NFC):
        fb1c[:, fc] = fb1[fc * 128 : (fc + 1) * 128]
        fw2[:, fc * C : (fc + 1) * C] = fW2[fc * 128 : (fc + 1) * 128, :].astype(BF)
    out.update(fw1=fw1, fb1c=fb1c, fw2=fw2, fb2=fb2.reshape(1, C).astype(BF))

    lm_W, lm_b = _np(params["lm_W"]), _np(params["lm_b"])
    lmw = np.zeros((128, NCH * V), BF)
    for ch in range(NCH):
        lmw[:, ch * V : (ch + 1) * V] = lm_W[ch * 128 : (ch + 1) * 128, :].astype(BF)
    out["lmw"] = lmw
    out["lmb"] = lm_b.reshape(1, V).astype(BF)

    out["tok"] = _np(params["tok"])  # [65, 384] f32
    out["pos"] = _np(params["pos"]).reshape(2, 128, C).transpose(1, 0, 2).reshape(128, 2 * C)
    out["iota"] = np.tile(np.arange(V, dtype=np.float32), (128, 1))
    out["iotac"] = np.arange(128, dtype=np.float32).reshape(128, 1)
    out["cmask"] = (np.arange(256)[None, :] >= np.arange(128)[:, None]).astype(BF)
    return out


def build(nc, L_RUN=L, NB=B_SH, debug=()):
    """Emit the model for NB batch units and L_RUN layers.

    debug: tap names among {"emb", "h1", "xattn0", "xlayer0", ...} added as
    extra outputs.
    """
    NTT_r = NB * 2                 # 128-token tiles
    ntok_r = NB * 256
    TBS = min(512, ntok_r)         # ffn/lm token block size
    NTB_r = ntok_r // TBS

    d = {}
    def din(name, shape, dt):
        d[name] = nc.dram_tensor(name, shape, dt, kind="ExternalInput")
    din("wqk", [L, 128, H * 3 * 128], BF16)
    din("bqk", [L, 128, H], F32)
    din("wv", [L, 128, NPAIR * 3 * 128], BF16)
    din("wp", [L, 128, 3 * C], BF16)
    din("bpb2", [L, 2, C], BF16)
    din("w1", [L, 128, 3 * NFC * 128], BF16)
    din("b1c", [L, 128, NFC], F32)
    din("w2", [L, 128, NFC * C], BF16)
    din("fw1", [128, 3 * NFC * 128], BF16)
    din("fb1c", [128, NFC], F32)
    din("fw2", [128, NFC * C], BF16)
    din("fb2", [1, C], BF16)
    din("lmw", [128, NCH * V], BF16)
    din("lmb", [1, V], BF16)
    din("tok", [V, C], F32)
    din("pos", [128, 2 * C], F32)
    din("iota", [128, V], F32)
    din("iotac", [128, 1], F32)
    din("cmask", [128, 256], BF16)
    din("idxf", [ntok_r, 1], F32)
    din("tgtf", [ntok_r, 1], F32)

    logits_out = nc.dram_tensor("logits", [ntok_r, V], F32, kind="ExternalOutput")
    loss_out = nc.dram_tensor("loss_sum", [1, 1], F32, kind="ExternalOutput")
    taps = {}
    for name in debug:
        dt = BF16 if name in ("h1",) else F32
        taps[name] = nc.dram_tensor("tap_" + name, [NTT_r * 128, C], dt,
                                    kind="ExternalOutput")

    with TileContext(nc) as tc:
        with (
            tc.tile_pool(name="const", bufs=1) as cpool,
            tc.tile_pool(name="wts", bufs=2) as wpool,
            tc.tile_pool(name="big", bufs=1) as bigpool,
            tc.tile_pool(name="work", bufs=3) as wk,
            tc.tile_pool(name="h1p", bufs=2) as h1p,
            tc.tile_pool(name="ps", bufs=2, space="PSUM") as psA,
            tc.tile_pool(name="psoz", bufs=4, space="PSUM") as psOZ,
            tc.tile_pool(name="psv", bufs=2, space="PSUM") as psV,
        ):
            # ---- constants -------------------------------------------------
            tok_sb = cpool.tile([V, C], F32, tag="tok")
            nc.sync.dma_start(tok_sb[:], d["tok"][:])
            pos_sb = cpool.tile([128, 2 * C], F32, tag="pos")
            nc.sync.dma_start(pos_sb[:], d["pos"][:])
            iota_sb = cpool.tile([128, V], F32, tag="iota")
            nc.sync.dma_start(iota_sb[:], d["iota"][:])
            iotac_sb = cpool.tile([128, 1], F32, tag="iotac")
            nc.sync.dma_start(iotac_sb[:], d["iotac"][:])
            cmask_sb = cpool.tile([128, 256], BF16, tag="cmask")
            nc.sync.dma_start(cmask_sb[:], d["cmask"][:])
            ones64 = cpool.tile([128, 64], BF16, tag="ones64")
            nc.vector.memset(ones64[:], 1.0)
            ones1 = cpool.tile([1, 128], BF16, tag="ones1")
            nc.vector.memset(ones1[:], 1.0)
            lmw_sb = cpool.tile([128, NCH * V], BF16, tag="lmw")
            nc.sync.dma_start(lmw_sb[:], d["lmw"][:])
            lmb_sb = cpool.tile([1, V], BF16, tag="lmb")
            nc.sync.dma_start(lmb_sb[:], d["lmb"][:])
            loss_acc = cpool.tile([128, 1], F32, tag="lacc")
            nc.vector.memset(loss_acc[:], 0.0)
            eps_sb = cpool.tile([128, 1], F32, tag="eps")
            nc.vector.memset(eps_sb[:], 1e-5)

            # persistent residual stream
            xs = [bigpool.tile([128, C], F32, tag=f"x{t}", name=f"x{t}") for t in range(NTT_r)]

            def dump_x(name):
                if name in taps:
                    for tt in range(NTT_r):
                        nc.sync.dma_start(
                            taps[name][tt * 128 : (tt + 1) * 128, :], xs[tt][:])

            # ---- embedding -------------------------------------------------
            for tt in range(NTT_r):
                idxbc = wk.tile([128, 128], F32, tag="idxbc")
                nc.sync.dma_start(
                    idxbc[:],
                    d["idxf"][tt * 128 : (tt + 1) * 128, 0:1]
                    .rearrange("t u -> u t").broadcast_to([128, 128]))
                ohT = wk.tile([128, 128], F32, tag="ohT")
                nc.vector.tensor_tensor(
                    ohT[:], idxbc[:], iotac_sb[:].broadcast_to([128, 128]),
                    ALU.is_equal)
                x0 = psA.tile([128, C], F32, tag="mmA")
                nc.tensor.matmul(x0[:], ohT[0:V, :], tok_sb[:], start=True, stop=True)
                nc.vector.tensor_tensor(
                    xs[tt][:], x0[:], pos_sb[:, (tt % 2) * C : (tt % 2 + 1) * C], ALU.add)
            dump_x("emb")

            def ln_tile(dst3, k, src_ap, apply_ln):
                """LN (or cast) one 128-token tile into slot k of an
                interleaved staging tile view dst3 [128, NCH, 4*128]."""
                out_ap = dst3[:, :, k * 128 : (k + 1) * 128]
                src3 = src_ap.rearrange("p (j q) -> p j q", j=NCH)
                if apply_ln:
                    st6 = wk.tile([128, 6], F32, tag="st6")
                    nc.vector.bn_stats(st6[:], src_ap)
                    mv = wk.tile([128, 2], F32, tag="mv")
                    nc.vector.bn_aggr(mv[:], st6[:])
                    sd = wk.tile([128, 1], F32, tag="sd")
                    nc.scalar.activation(sd[:], mv[:, 1:2], AF.Sqrt, bias=eps_sb[:])
                    rstd = wk.tile([128, 1], F32, tag="rstd")
                    nc.vector.reciprocal(rstd[:], sd[:])
                    nmr = wk.tile([128, 1], F32, tag="nmr")
                    nc.vector.scalar_tensor_tensor(
                        nmr[:], mv[:, 0:1], -1.0, rstd[:], ALU.mult, ALU.mult)
                    nc.scalar.activation(
                        out_ap, src3, AF.Identity, bias=nmr[:], scale=rstd[:])
                else:
                    nc.scalar.copy(out_ap, src3)

            def layer_norm_T(apply_ln=True, tapname=None):
                """LN(x) (or plain bf16 cast) -> batched DMA-transpose (4
                token tiles per descriptor) into hT chunk views."""
                hT_all = bigpool.tile([128, NCH * ntok_r], BF16, tag="hTall", name="hTall")
                hT = [hT_all[:, j * ntok_r : (j + 1) * ntok_r] for j in range(NCH)]
                hT3 = hT_all[:].rearrange("p (j t) -> p j t", j=NCH)
                for tt in range(NTT_r):
                    hb = wk.tile([128, C], BF16, tag="hb")
                    hb3 = hb[:].rearrange("p (j q) -> p j q", j=NCH)
                    ln_tile(hb3, 0, xs[tt][:], apply_ln)
                    if tapname is not None and tapname in taps:
                        nc.sync.dma_start(
                            taps[tapname][tt * 128 : (tt + 1) * 128, :], hb[:])
                    teng = nc.sync if tt % 2 == 0 else nc.scalar
                    teng.dma_start_transpose(
                        hT3[:, :, tt * 128 : (tt + 1) * 128], hb[:])
                return hT

            # ---- transformer layers ---------------------------------------
            for l in range(L_RUN):
                wqk_sb = wpool.tile([128, H * 3 * 128], BF16, tag="wqk")
                nc.sync.dma_start(wqk_sb[:], d["wqk"][l])
                bqk_sb = wpool.tile([128, H], F32, tag="bqk")
                nc.sync.dma_start(bqk_sb[:], d["bqk"][l])
                wv_sb = wpool.tile([128, NPAIR * 3 * 128], BF16, tag="wv")
                nc.sync.dma_start(wv_sb[:], d["wv"][l])
                wp_sb = wpool.tile([128, 3 * C], BF16, tag="wp")
                nc.sync.dma_start(wp_sb[:], d["wp"][l])
                bp_sb = wpool.tile([1, C], BF16, tag="bprow")
                nc.sync.dma_start(bp_sb[:], d["bpb2"][l, 0:1, :])
                b2_sb = wpool.tile([1, C], BF16, tag="b2row")
                nc.sync.dma_start(b2_sb[:], d["bpb2"][l, 1:2, :])
                w1_sb = wpool.tile([128, 3 * NFC * 128], BF16, tag="w1", bufs=1)
                nc.sync.dma_start(w1_sb[:], d["w1"][l])
                b1_sb = wpool.tile([128, NFC], F32, tag="b1c")
                nc.sync.dma_start(b1_sb[:], d["b1c"][l])
                w2_sb = wpool.tile([128, NFC * C], BF16, tag="w2", bufs=1)
                nc.sync.dma_start(w2_sb[:], d["w2"][l])

                hT = layer_norm_T(apply_ln=True, tapname="h1" if l == 0 else None)

                GRP = 2 if NB % 2 == 0 else 1  # batches per qk/v group
                NG = GRP * 256
                for g in range(NB // GRP):
                    t0 = g * NG
                    # v for all pairs, N=384 per token chunk
                    v_sb = []
                    for tch in range(2 * GRP):
                        vps = psV.tile([128, NPAIR * 128], F32, tag="v")
                        for ch in range(NCH):
                            nc.tensor.matmul(
                                vps[:],
                                hT[ch][:, t0 + tch * 128 : t0 + (tch + 1) * 128],
                                wv_sb[:, ch * NPAIR * 128 : (ch + 1) * NPAIR * 128],
                                start=(ch == 0), stop=(ch == NCH - 1))
                        vsb = wk.tile([128, NPAIR * 128], BF16, tag=f"vsb{tch}")
                        nc.scalar.copy(vsb[:], vps[:])
                        v_sb.append(vsb)

                    # qk for all heads, N=512 over the group
                    qk_sb = []
                    ksp_t = []
                    for h in range(H):
                        qk = psA.tile([128, NG], F32, tag="mmA")
                        for ch in range(NCH):
                            nc.tensor.matmul(
                                qk[:],
                                wqk_sb[:, (h * 3 + ch) * 128 : (h * 3 + ch + 1) * 128],
                                hT[ch][:, t0 : t0 + NG],
                                start=(ch == 0), stop=(ch == NCH - 1))
                        qksb = wk.tile([128, NG], BF16, tag=f"qksb{h}", bufs=2)
                        nc.vector.tensor_scalar(
                            qksb[:], qk[:], bqk_sb[:, h : h + 1], None, ALU.add)
                        ksp = wk.tile([64, NG], BF16, tag=f"ksp{h}", bufs=2)
                        nc.gpsimd.dma_start(ksp[:], qksb[64:128, :])
                        qk_sb.append(qksb)
                        ksp_t.append(ksp)

                    for bi in range(GRP):
                        b = g * GRP + bi
                        q0 = bi * 256
                        ostks = []
                        for pr in range(NPAIR):
                            e_t = []
                            for hh in range(2):
                                h = 2 * pr + hh
                                qksb, ksp = qk_sb[h], ksp_t[h]
                                sc = psA.tile([128, 384], F32, tag="mmA")
                                nc.tensor.matmul(
                                    sc[:, 0:256], ksp[:, q0 : q0 + 128],
                                    qksb[0:64, q0 : q0 + 256],
                                    start=True, stop=True)
                                nc.tensor.matmul(
                                    sc[:, 256:384], ksp[:, q0 + 128 : q0 + 256],
                                    qksb[0:64, q0 + 128 : q0 + 256],
                                    start=True, stop=True)
                                e = wk.tile([128, 384], BF16, tag="e")
                                nc.scalar.activation(e[:, 0:256], sc[:, 0:256], AF.Exp, scale=SCALE)
                                nc.scalar.activation(e[:, 256:384], sc[:, 256:384], AF.Exp, scale=SCALE)
                                nc.vector.tensor_tensor(e[:, 0:256], e[:, 0:256], cmask_sb[:], ALU.mult)
                                nc.vector.tensor_tensor(
                                    e[:, 256:384], e[:, 256:384], cmask_sb[:, 0:128], ALU.mult)
                                e_t.append(e)

                            opair = psOZ.tile([128, 256], F32, tag="oz")
                            zpair = psOZ.tile([128, 256], F32, tag="oz")
                            v0 = v_sb[2 * bi]
                            v1 = v_sb[2 * bi + 1]
                            for hh in range(2):
                                e = e_t[hh]
                                r0, r1 = hh * 64, (hh + 1) * 64
                                c0 = pr * 128 + hh * 64
                                nc.tensor.matmul(
                                    opair[r0:r1, 0:256], v0[:, c0 : c0 + 64],
                                    e[:, 0:256], start=True, stop=False)
                                nc.tensor.matmul(
                                    opair[r0:r1, 128:256], v1[:, c0 : c0 + 64],
                                    e[:, 256:384], start=False, stop=True)
                                nc.tensor.matmul(
                                    zpair[r0:r1, 0:256], ones64[:, 0:64],
                                    e[:, 0:256], start=True, stop=False)
                                nc.tensor.matmul(
                                    zpair[r0:r1, 128:256], ones64[:, 0:64],
                                    e[:, 256:384], start=False, stop=True)
                            rz = wk.tile([128, 256], BF16, tag="rz")
                            with nc.allow_low_precision(reason="softmax denom bf16"):
                                nc.vector.reciprocal(rz[:], zpair[:])
                            ostk = wk.tile([128, 256], BF16, tag=f"ostk{pr}")
                            nc.vector.tensor_tensor(ostk[:], opair[:], rz[:], ALU.mult)
                            ostks.append(ostk)

                        for th in range(2):
                            tt = 2 * b + th
                            dx = psA.tile([128, C], F32, tag="mmA")
                            for pr in range(NPAIR):
                                nc.tensor.matmul(
                                    dx[:],
                                    ostks[pr][:, th * 128 : (th + 1) * 128],
                                    wp_sb[:, pr * C : (pr + 1) * C],
                                    start=(pr == 0), stop=False)
                            nc.tensor.matmul(dx[:], ones1[:], bp_sb[0:1, :],
                                             start=False, stop=True)
                            nc.vector.tensor_tensor(xs[tt][:], xs[tt][:], dx[:], ALU.add)
                dump_x(f"xattn{l}")

                h2T = layer_norm_T(apply_ln=True)
                for tb in range(NTB_r):
                    h1s = []
                    for fc in range(NFC):
                        h1 = psA.tile([128, TBS], F32, tag="mmA")
                        for ch in range(NCH):
                            nc.tensor.matmul(
                                h1[:],
                                w1_sb[:, (ch * NFC + fc) * 128 : (ch * NFC + fc + 1) * 128],
                                h2T[ch][:, tb * TBS : (tb + 1) * TBS],
                                start=(ch == 0), stop=(ch == NCH - 1))
                        h1sb = h1p.tile([128, TBS], BF16, tag=f"h1_{fc}")
                        nc.scalar.activation(h1sb[:], h1[:], AF.Relu,
                                             bias=b1_sb[:, fc : fc + 1])
                        h1s.append(h1sb)
                    for t4 in range(TBS // 128):
                        tt = tb * (TBS // 128) + t4
                        dx = psA.tile([128, C], F32, tag="mmA")
                        for fc in range(NFC):
                            nc.tensor.matmul(
                                dx[:],
                                h1s[fc][:, t4 * 128 : (t4 + 1) * 128],
                                w2_sb[:, fc * C : (fc + 1) * C],
                                start=(fc == 0), stop=False)
                        nc.tensor.matmul(dx[:], ones1[:], b2_sb[0:1, :],
                                         start=False, stop=True)
                        nc.vector.tensor_tensor(xs[tt][:], xs[tt][:], dx[:], ALU.add)
                dump_x(f"xlayer{l}")

            # ---- final FFN (no LN, no residual) + LM head + loss ----------
            fw1_sb = wpool.tile([128, 3 * NFC * 128], BF16, tag="w1", bufs=1)
            nc.sync.dma_start(fw1_sb[:], d["fw1"][:])
            fb1_sb = wpool.tile([128, NFC], F32, tag="b1c")
            nc.sync.dma_start(fb1_sb[:], d["fb1c"][:])
            fw2_sb = wpool.tile([128, NFC * C], BF16, tag="w2", bufs=1)
            nc.sync.dma_start(fw2_sb[:], d["fw2"][:])
            fb2_sb = wpool.tile([1, C], BF16, tag="b2row")
            nc.sync.dma_start(fb2_sb[:], d["fb2"][:])

            xT = layer_norm_T(apply_ln=False)
            for tb in range(NTB_r):
                h1s = []
                for fc in range(NFC):
                    h1 = psA.tile([128, TBS], F32, tag="mmA")
                    for ch in range(NCH):
                        nc.tensor.matmul(
                            h1[:],
                            fw1_sb[:, (ch * NFC + fc) * 128 : (ch * NFC + fc + 1) * 128],
                            xT[ch][:, tb * TBS : (tb + 1) * TBS],
                            start=(ch == 0), stop=(ch == NCH - 1))
                    h1sb = h1p.tile([128, TBS], BF16, tag=f"h1_{fc}")
                    nc.scalar.activation(h1sb[:], h1[:], AF.Relu,
                                         bias=fb1_sb[:, fc : fc + 1])
                    h1s.append(h1sb)
                yTb_all = wk.tile([128, NCH * TBS], BF16, tag="yTball", name="yTball")
                yTb = [yTb_all[:, j * TBS : (j + 1) * TBS] for j in range(NCH)]
                yTb3 = yTb_all[:].rearrange("p (j t) -> p j t", j=NCH)

                for t4 in range(TBS // 128):
                    tt = tb * (TBS // 128) + t4
                    dx = psA.tile([128, C], F32, tag="mmA")
                    for fc in range(NFC):
                        nc.tensor.matmul(
                            dx[:],
                            h1s[fc][:, t4 * 128 : (t4 + 1) * 128],
                            fw2_sb[:, fc * C : (fc + 1) * C],
                            start=(fc == 0), stop=False)
                    nc.tensor.matmul(dx[:], ones1[:], fb2_sb[0:1, :],
                                     start=False, stop=True)
                    yb = wk.tile([128, C], BF16, tag="hb")
                    nc.scalar.copy(yb[:], dx[:])
                    teng = nc.sync if t4 % 2 == 0 else nc.scalar
                    teng.dma_start_transpose(
                        yTb3[:, :, t4 * 128 : (t4 + 1) * 128], yb[:])

                for t4 in range(TBS // 128):
                    tt = tb * (TBS // 128) + t4
                    lgn = psA.tile([128, V], F32, tag="mmA")
                    for ch in range(NCH):
                        nc.tensor.matmul(
                            lgn[:], yTb[ch][:, t4 * 128 : (t4 + 1) * 128],
                            lmw_sb[:, ch * V : (ch + 1) * V],
                            start=(ch == 0), stop=False)
                    nc.tensor.matmul(lgn[:], ones1[:], lmb_sb[0:1, :],
                                     start=False, stop=True)
                    lg = wk.tile([128, V], F32, tag="lg")
                    nc.scalar.copy(lg[:], lgn[:])
                    nc.gpsimd.dma_start(logits_out[tt * 128 : (tt + 1) * 128, :], lg[:])
                    tgtc = wk.tile([128, 1], F32, tag="tgtc")
                    nc.gpsimd.dma_start(tgtc[:], d["tgtf"][tt * 128 : (tt + 1) * 128, :])
                    ohg = wk.tile([128, V], F32, tag="ohg")
                    nc.vector.tensor_scalar(ohg[:], iota_sb[:], tgtc[:], None, ALU.is_equal)
                    mx = wk.tile([128, 1], F32, tag="mx")
                    nc.vector.tensor_reduce(mx[:], lg[:], AX.X, ALU.max)
                    nmx = wk.tile([128, 1], F32, tag="nmx")
                    nc.vector.tensor_scalar_mul(nmx[:], mx[:], -1.0)
                    esc = wk.tile([128, V], F32, tag="esc")
                    zc = wk.tile([128, 1], F32, tag="zc")
                    nc.scalar.activation(esc[:], lg[:], AF.Exp, bias=nmx[:], accum_out=zc[:])
                    lnz = wk.tile([128, 1], F32, tag="lnz")
                    nc.scalar.activation(lnz[:], zc[:], AF.Ln)
                    tscr = wk.tile([128, V], F32, tag="tscr")
                    tgl = wk.tile([128, 1], F32, tag="tgl")
                    nc.vector.scalar_tensor_tensor(
                        tscr[:], lg[:], 1.0, ohg[:], ALU.mult, ALU.mult, accum_out=tgl[:])
                    lt = wk.tile([128, 1], F32, tag="lt")
                    nc.vector.scalar_tensor_tensor(
                        lt[:], lnz[:], mx[:], tgl[:], ALU.add, ALU.subtract)
                    nc.vector.tensor_tensor(loss_acc[:], loss_acc[:], lt[:], ALU.add)
            lsum = cpool.tile([1, 1], F32, tag="lsum")
            nc.gpsimd.tensor_reduce(lsum[:], loss_acc[:], AX.C, ALU.add)
            nc.sync.dma_start(loss_out[:], lsum[:])
    return d


def make_in_map(pp, idx_sh, tgt_sh):
    m = dict(pp)
    ntok = idx_sh.size
    m["idxf"] = idx_sh.reshape(ntok, 1).astype(np.float32)
    m["tgtf"] = tgt_sh.reshape(ntok, 1).astype(np.float32)
    return m


def kernel(params=None, idx=None, targets=None):
    pp = prep_params(params)
    idx = _np(idx, np.int64)
    targets = _np(targets, np.int64)

    nc = bass.Bass()
    build(nc)

    in_maps = [
        make_in_map(pp, idx[c * B_SH : (c + 1) * B_SH], targets[c * B_SH : (c + 1) * B_SH])
        for c in range(NCORES)
    ]
    res = run_bass_kernel_spmd(nc, in_maps, list(range(NCORES))).results

    logits = np.concatenate(
        [r["logits"].reshape(B_SH, T, V) for r in res], axis=0).astype(np.float32)
    loss = np.float32(sum(float(r["loss_sum"][0, 0]) for r in res) / (B * T))
    return logits, loss


# revision 18
# speedup vs baseline: 1.7488x; 1.1080x over previous
"""Bass/Trainium2 kernel for nn_BigramLanguageModel (6-layer dense
transformer, B=128 T=256 C=384 H=6 V=65), data-parallel over batch on 8
NeuronCores.

Layout strategy (per core, 16 batches = 4096 tokens):
- residual stream x kept fp32 in natural [token, channel] layout
- LayerNorm gamma/beta folded into the weights host-side; on-device LN is
  just (x - mean) * rstd, applied by ScalarE with per-partition scale/bias
- activations transposed to [channel, token] with bf16 DMA-transposes so
  every matmul contracts over the partition dim
- attention softmax computed transposed ([s, q]); the denominator comes
  from an all-ones stationary-operand matmul which yields Z already
  broadcast across partitions; heads processed in pairs sharing the 128
  partitions; causal structure skips the dead quarter of the matmuls
- all matmuls bf16 with fp32 PSUM accumulation (embedding matmul in fp32)
"""
import numpy as np
import ml_dtypes

import concourse.bass as bass
import concourse.mybir as mybir
from concourse.bass_utils import run_bass_kernel_spmd
from concourse.tile import TileContext
from concourse.vector_clock import ScopedClock

F32 = mybir.dt.float32
BF16 = mybir.dt.bfloat16
AF = mybir.ActivationFunctionType
ALU = mybir.AluOpType
AX = mybir.AxisListType

V, C, BLOCK, H, L, D = 65, 384, 256, 6, 6, 64
B, T = 128, 256
F = 4 * C  # 1536
SCALE = C ** -0.5
NCORES = 8
B_SH = B // NCORES          # 16 batches per core
NTOK = B_SH * T             # 4096 tokens per core
NCH = C // 128              # 3 channel chunks
NFC = F // 128              # 12 ffn chunks
NPAIR = H // 2              # 3 head pairs
BF = ml_dtypes.bfloat16

# ---------------------------------------------------------------------------
# walrus in this container rejects >1 semaphore wait per instruction; peel
# extras onto same-engine nops (body commit hook + tail drain).
_MAXW = 1
_orig_add_instruction = TileContext._add_instruction


def _patched_add_instruction(self, inst):
    si = inst.sync_info
    if si is not None and si.on_wait and len(si.on_wait) > _MAXW:
        waits = list(si.on_wait)
        extra, keep = waits[:-_MAXW], waits[-_MAXW:]
        for i in range(0, len(extra), _MAXW):
            nop = mybir.InstNoOp(
                name=self.nc.get_next_instruction_name(),
                engine=inst.engine,
                sync_info=mybir.SyncInfo(on_wait=extra[i : i + _MAXW], on_update=[]),
                bass_nofuse=True,
            )
            _orig_add_instruction(self, nop)
        inst.sync_info = mybir.SyncInfo(on_wait=keep, on_update=list(si.on_update or []))
    _orig_add_instruction(self, inst)


def _patched_drain_and_barrier(self, tick_clock, wait_clock):
    carrier = self.nc.sync.nop()
    wait_clock.add_sem_waits(carrier.ins, ScopedClock({None: tick_clock.global_clock}))
    si = carrier.ins.sync_info
    waits = list(si.on_wait) if si is not None and si.on_wait else []
    if len(waits) > _MAXW:
        si.on_wait = waits[:_MAXW]
        for i in range(_MAXW, len(waits), _MAXW):
            n = self.nc.sync.nop()
            n.ins.sync_info = mybir.SyncInfo(on_wait=waits[i : i + _MAXW], on_update=[])
    self.nc.sync.drain()
    self.nc.all_engine_barrier()
    popped = self.nc._tile_sem_poison_stack.pop()
    assert popped is self._sem_poison
    self.nc.clear_and_free_semaphores(list(self.sems.allocated().values()))
    self.nc.all_engine_barrier()


TileContext._add_instruction = _patched_add_instruction
TileContext._drain_and_barrier = _patched_drain_and_barrier
# ---------------------------------------------------------------------------


def _np(x, dt=np.float32):
    return np.ascontiguousarray(np.asarray(x, dtype=dt))


def prep_params(params):
    """Fold LN gamma/beta into weights; pack per-layer SBUF images."""
    out = {}
    wqk = np.zeros((L, 128, H * 3 * 128), BF)
    bqk = np.zeros((L, 128, H), np.float32)
    wv = np.zeros((L, 128, NPAIR * 3 * 128), BF)
    wp = np.zeros((L, 128, 3 * C), BF)
    bpb2 = np.zeros((L, 2, C), BF)
    w1 = np.zeros((L, 128, 3 * NFC * 128), BF)
    b1c = np.zeros((L, 128, NFC), np.float32)
    w2 = np.zeros((L, 128, NFC * C), BF)
    for l, p in enumerate(params["blocks"]):
        g1, be1 = _np(p["ln1_g"]), _np(p["ln1_b"])
        g2, be2 = _np(p["ln2_g"]), _np(p["ln2_b"])
        Wq, Wk, Wv_ = _np(p["Wq"]), _np(p["Wk"]), _np(p["Wv"])  # [H, C, D]
        Wp, bp = _np(p["Wp"]), _np(p["bp"])
        W1, b1 = _np(p["W1"]), _np(p["b1"])
        W2, b2 = _np(p["W2"]), _np(p["b2"])
        gWq = g1[None, :, None] * Wq
        gWk = g1[None, :, None] * Wk
        gWv = g1[None, :, None] * Wv_
        bWq = np.einsum("c,hcd->hd", be1, Wq)
        bWk = np.einsum("c,hcd->hd", be1, Wk)
        bWv = np.einsum("c,hcd->hd", be1, Wv_)  # folded into proj bias
        for h in range(H):
            for ch in range(NCH):
                blk = np.concatenate(
                    [gWq[h, ch * 128 : (ch + 1) * 128, :],
                     gWk[h, ch * 128 : (ch + 1) * 128, :]], axis=1)
                wqk[l, :, (h * 3 + ch) * 128 : (h * 3 + ch + 1) * 128] = blk.astype(BF)
            bqk[l, 0:64, h] = bWq[h]
            bqk[l, 64:128, h] = bWk[h]
        for pr in range(NPAIR):
            for ch in range(NCH):
                blk = np.concatenate(
                    [gWv[2 * pr, ch * 128 : (ch + 1) * 128, :],
                     gWv[2 * pr + 1, ch * 128 : (ch + 1) * 128, :]], axis=1)
                wv[l, :, (ch * NPAIR + pr) * 128 : (ch * NPAIR + pr + 1) * 128] = blk.astype(BF)
        for ch in range(NCH):
            wp[l, :, ch * C : (ch + 1) * C] = Wp[ch * 128 : (ch + 1) * 128, :].astype(BF)
        bp_adj = bp + bWv.reshape(C) @ Wp  # softmax rows sum to 1
        bpb2[l, 0, :] = bp_adj.astype(BF)
        bpb2[l, 1, :] = b2.astype(BF)
        gW1 = g2[:, None] * W1
        bW1 = be2 @ W1
        for ch in range(NCH):
            for fc in range(NFC):
                w1[l, :, (ch * NFC + fc) * 128 : (ch * NFC + fc + 1) * 128] = gW1[
                    ch * 128 : (ch + 1) * 128, fc * 128 : (fc + 1) * 128].astype(BF)
        for fc in range(NFC):
            b1c[l, :, fc] = (b1 + bW1)[fc * 128 : (fc + 1) * 128]
            w2[l, :, fc * C : (fc + 1) * C] = W2[fc * 128 : (fc + 1) * 128, :].astype(BF)
    out.update(wqk=wqk, bqk=bqk, wv=wv, wp=wp, bpb2=bpb2, w1=w1, b1c=b1c, w2=w2)

    fW1, fb1 = _np(params["fW1"]), _np(params["fb1"])
    fW2, fb2 = _np(params["fW2"]), _np(params["fb2"])
    fw1 = np.zeros((128, 3 * NFC * 128), BF)
    fb1c = np.zeros((128, NFC), np.float32)
    fw2 = np.zeros((128, NFC * C), BF)
    for ch in range(NCH):
        for fc in range(NFC):
            fw1[:, (ch * NFC + fc) * 128 : (ch * NFC + fc + 1) * 128] = fW1[
                ch * 128 : (ch + 1) * 128, fc * 128 : (fc + 1) * 128].astype(BF)
    for fc in range(NFC):
        fb1c[:, fc] = fb1[fc * 128 : (fc + 1) * 128]
        fw2[:, fc * C : (fc + 1) * C] = fW2[fc * 128 : (fc + 1) * 128, :].astype(BF)
    out.update(fw1=fw1, fb1c=fb1c, fw2=fw2, fb2=fb2.reshape(1, C).astype(BF))

    lm_W, lm_b = _np(params["lm_W"]), _np(params["lm_b"])
    lmw = np.zeros((128, NCH * V), BF)
    for ch in range(NCH):
        lmw[:, ch * V : (ch + 1) * V] = lm_W[ch * 128 : (ch + 1) * 128, :].astype(BF)
    out["lmw"] = lmw
    out["lmb"] = lm_b.reshape(1, V).astype(BF)

    out["tok"] = _np(params["tok"])  # [65, 384] f32
    out["pos"] = _np(params["pos"]).reshape(2, 128, C).transpose(1, 0, 2).reshape(128, 2 * C)
    out["iota"] = np.tile(np.arange(V, dtype=np.float32), (128, 1))
    out["iotac"] = np.arange(128, dtype=np.float32).reshape(128, 1)
    out["cmask"] = (np.arange(256)[None, :] >= np.arange(128)[:, None]).astype(BF)
    return out


def build(nc, L_RUN=L, NB=B_SH, debug=()):
    """Emit the model for NB batch units and L_RUN layers.

    debug: tap names among {"emb", "h1", "xattn0", "xlayer0", ...} added as
    extra outputs.
    """
    NTT_r = NB * 2                 # 128-token tiles
    ntok_r = NB * 256
    TBS = min(512, ntok_r)         # ffn/lm token block size
    NTB_r = ntok_r // TBS

    d = {}
    def din(name, shape, dt):
        d[name] = nc.dram_tensor(name, shape, dt, kind="ExternalInput")
    din("wqk", [L, 128, H * 3 * 128], BF16)
    din("bqk", [L, 128, H], F32)
    din("wv", [L, 128, NPAIR * 3 * 128], BF16)
    din("wp", [L, 128, 3 * C], BF16)
    din("bpb2", [L, 2, C], BF16)
    din("w1", [L, 128, 3 * NFC * 128], BF16)
    din("b1c", [L, 128, NFC], F32)
    din("w2", [L, 128, NFC * C], BF16)
    din("fw1", [128, 3 * NFC * 128], BF16)
    din("fb1c", [128, NFC], F32)
    din("fw2", [128, NFC * C], BF16)
    din("fb2", [1, C], BF16)
    din("lmw", [128, NCH * V], BF16)
    din("lmb", [1, V], BF16)
    din("tok", [V, C], F32)
    din("pos", [128, 2 * C], F32)
    din("iota", [128, V], F32)
    din("iotac", [128, 1], F32)
    din("cmask", [128, 256], BF16)
    din("idxf", [ntok_r, 1], F32)
    din("tgtf", [ntok_r, 1], F32)

    logits_out = nc.dram_tensor("logits", [ntok_r, V], F32, kind="ExternalOutput")
    loss_out = nc.dram_tensor("loss_sum", [1, 1], F32, kind="ExternalOutput")
    taps = {}
    for name in debug:
        dt = BF16 if name in ("h1",) else F32
        taps[name] = nc.dram_tensor("tap_" + name, [NTT_r * 128, C], dt,
                                    kind="ExternalOutput")

    with TileContext(nc) as tc:
        with (
            tc.tile_pool(name="const", bufs=1) as cpool,
            tc.tile_pool(name="wts", bufs=2) as wpool,
            tc.tile_pool(name="big", bufs=1) as bigpool,
            tc.tile_pool(name="work", bufs=3) as wk,
            tc.tile_pool(name="h1p", bufs=2) as h1p,
            tc.tile_pool(name="ps", bufs=3, space="PSUM") as psA,
            tc.tile_pool(name="psoz", bufs=3, space="PSUM") as psOZ,
            tc.tile_pool(name="psv", bufs=2, space="PSUM") as psV,
        ):
            # ---- constants -------------------------------------------------
            tok_sb = cpool.tile([V, C], F32, tag="tok")
            nc.sync.dma_start(tok_sb[:], d["tok"][:])
            pos_sb = cpool.tile([128, 2 * C], F32, tag="pos")
            nc.sync.dma_start(pos_sb[:], d["pos"][:])
            iota_sb = cpool.tile([128, V], F32, tag="iota")
            nc.sync.dma_start(iota_sb[:], d["iota"][:])
            iotac_sb = cpool.tile([128, 1], F32, tag="iotac")
            nc.sync.dma_start(iotac_sb[:], d["iotac"][:])
            cmask_sb = cpool.tile([128, 256], BF16, tag="cmask")
            nc.sync.dma_start(cmask_sb[:], d["cmask"][:])
            ones64 = cpool.tile([128, 64], BF16, tag="ones64")
            nc.vector.memset(ones64[:], 1.0)
            ones1 = cpool.tile([1, 128], BF16, tag="ones1")
            nc.vector.memset(ones1[:], 1.0)
            lmw_sb = cpool.tile([128, NCH * V], BF16, tag="lmw")
            nc.sync.dma_start(lmw_sb[:], d["lmw"][:])
            lmb_sb = cpool.tile([1, V], BF16, tag="lmb")
            nc.sync.dma_start(lmb_sb[:], d["lmb"][:])
            loss_acc = cpool.tile([128, 1], F32, tag="lacc")
            nc.vector.memset(loss_acc[:], 0.0)
            eps_sb = cpool.tile([128, 1], F32, tag="eps")
            nc.vector.memset(eps_sb[:], 1e-5)

            # persistent residual stream
            xs = [bigpool.tile([128, C], F32, tag=f"x{t}", name=f"x{t}") for t in range(NTT_r)]

            def dump_x(name):
                if name in taps:
                    for tt in range(NTT_r):
                        nc.sync.dma_start(
                            taps[name][tt * 128 : (tt + 1) * 128, :], xs[tt][:])

            # ---- embedding -------------------------------------------------
            for tt in range(NTT_r):
                idxbc = wk.tile([128, 128], F32, tag="idxbc")
                nc.sync.dma_start(
                    idxbc[:],
                    d["idxf"][tt * 128 : (tt + 1) * 128, 0:1]
                    .rearrange("t u -> u t").broadcast_to([128, 128]))
                ohT = wk.tile([128, 128], F32, tag="ohT")
                nc.vector.tensor_tensor(
                    ohT[:], idxbc[:], iotac_sb[:].broadcast_to([128, 128]),
                    ALU.is_equal)
                x0 = psA.tile([128, C], F32, tag="mmA")
                nc.tensor.matmul(x0[:], ohT[0:V, :], tok_sb[:], start=True, stop=True)
                nc.vector.tensor_tensor(
                    xs[tt][:], x0[:], pos_sb[:, (tt % 2) * C : (tt % 2 + 1) * C], ALU.add)
            dump_x("emb")

            def ln_tile(dst3, k, src_ap, apply_ln):
                """LN (or cast) one 128-token tile into slot k of an
                interleaved staging tile view dst3 [128, NCH, 4*128]."""
                out_ap = dst3[:, :, k * 128 : (k + 1) * 128]
                src3 = src_ap.rearrange("p (j q) -> p j q", j=NCH)
                if apply_ln:
                    st6 = wk.tile([128, 6], F32, tag="st6")
                    nc.vector.bn_stats(st6[:], src_ap)
                    mv = wk.tile([128, 2], F32, tag="mv")
                    nc.vector.bn_aggr(mv[:], st6[:])
                    sd = wk.tile([128, 1], F32, tag="sd")
                    nc.scalar.activation(sd[:], mv[:, 1:2], AF.Sqrt, bias=eps_sb[:])
                    rstd = wk.tile([128, 1], F32, tag="rstd")
                    nc.vector.reciprocal(rstd[:], sd[:])
                    nmr = wk.tile([128, 1], F32, tag="nmr")
                    nc.vector.scalar_tensor_tensor(
                        nmr[:], mv[:, 0:1], -1.0, rstd[:], ALU.mult, ALU.mult)
                    nc.scalar.activation(
                        out_ap, src3, AF.Identity, bias=nmr[:], scale=rstd[:])
                else:
                    nc.scalar.copy(out_ap, src3)

            def layer_norm_T(apply_ln=True, tapname=None):
                """LN(x) (or plain bf16 cast) -> batched DMA-transpose (4
                token tiles per descriptor) into hT chunk views."""
                hT_all = bigpool.tile([128, NCH * ntok_r], BF16, tag="hTall", name="hTall")
                hT = [hT_all[:, j * ntok_r : (j + 1) * ntok_r] for j in range(NCH)]
                hT3 = hT_all[:].rearrange("p (j t) -> p j t", j=NCH)
                for tt in range(NTT_r):
                    hb = wk.tile([128, C], BF16, tag="hb")
                    hb3 = hb[:].rearrange("p (j q) -> p j q", j=NCH)
                    ln_tile(hb3, 0, xs[tt][:], apply_ln)
                    if tapname is not None and tapname in taps:
                        nc.sync.dma_start(
                            taps[tapname][tt * 128 : (tt + 1) * 128, :], hb[:])
                    teng = nc.sync if tt % 2 == 0 else nc.scalar
                    teng.dma_start_transpose(
                        hT3[:, :, tt * 128 : (tt + 1) * 128], hb[:])
                return hT

            # ---- transformer layers ---------------------------------------
            for l in range(L_RUN):
                wqk_sb = wpool.tile([128, H * 3 * 128], BF16, tag="wqk")
                nc.sync.dma_start(wqk_sb[:], d["wqk"][l])
                bqk_sb = wpool.tile([128, H], F32, tag="bqk")
                nc.sync.dma_start(bqk_sb[:], d["bqk"][l])
                wv_sb = wpool.tile([128, NPAIR * 3 * 128], BF16, tag="wv")
                nc.sync.dma_start(wv_sb[:], d["wv"][l])
                wp_sb = wpool.tile([128, 3 * C], BF16, tag="wp")
                nc.sync.dma_start(wp_sb[:], d["wp"][l])
                bp_sb = wpool.tile([1, C], BF16, tag="bprow")
                nc.sync.dma_start(bp_sb[:], d["bpb2"][l, 0:1, :])
                b2_sb = wpool.tile([1, C], BF16, tag="b2row")
                nc.sync.dma_start(b2_sb[:], d["bpb2"][l, 1:2, :])
                w1_sb = wpool.tile([128, 3 * NFC * 128], BF16, tag="w1", bufs=1)
                nc.sync.dma_start(w1_sb[:], d["w1"][l])
                b1_sb = wpool.tile([128, NFC], F32, tag="b1c")
                nc.sync.dma_start(b1_sb[:], d["b1c"][l])
                w2_sb = wpool.tile([128, NFC * C], BF16, tag="w2", bufs=1)
                nc.sync.dma_start(w2_sb[:], d["w2"][l])

                hT = layer_norm_T(apply_ln=True, tapname="h1" if l == 0 else None)

                GRP = 2 if NB % 2 == 0 else 1  # batches per qk/v group
                NG = GRP * 256
                for g in range(NB // GRP):
                    t0 = g * NG
                    # v for all pairs, N=384 per token chunk
                    v_sb = []
                    for tch in range(2 * GRP):
                        vps = psV.tile([128, NPAIR * 128], F32, tag="v")
                        for ch in range(NCH):
                            nc.tensor.matmul(
                                vps[:],
                                hT[ch][:, t0 + tch * 128 : t0 + (tch + 1) * 128],
                                wv_sb[:, ch * NPAIR * 128 : (ch + 1) * NPAIR * 128],
                                start=(ch == 0), stop=(ch == NCH - 1))
                        vsb = wk.tile([128, NPAIR * 128], BF16, tag=f"vsb{tch}")
                        nc.scalar.copy(vsb[:], vps[:])
                        v_sb.append(vsb)

                    # qk for all heads, N=512 over the group
                    qk_sb = []
                    ksp_t = []
                    for h in range(H):
                        qk = psA.tile([128, NG], F32, tag="mmA")
                        for ch in range(NCH):
                            nc.tensor.matmul(
                                qk[:],
                                wqk_sb[:, (h * 3 + ch) * 128 : (h * 3 + ch + 1) * 128],
                                hT[ch][:, t0 : t0 + NG],
                                start=(ch == 0), stop=(ch == NCH - 1))
                        qksb = wk.tile([128, NG], BF16, tag=f"qksb{h}", bufs=2)
                        nc.vector.tensor_scalar(
                            qksb[:], qk[:], bqk_sb[:, h : h + 1], None, ALU.add)
                        ksp = wk.tile([64, NG], BF16, tag=f"ksp{h}", bufs=2)
                        nc.gpsimd.dma_start(ksp[:], qksb[64:128, :])
                        qk_sb.append(qksb)
                        ksp_t.append(ksp)

                    for bi in range(GRP):
                        b = g * GRP + bi
                        q0 = bi * 256
                        ostks = []
                        for pr in range(NPAIR):
                            e_t = []
                            for hh in range(2):
                                h = 2 * pr + hh
                                qksb, ksp = qk_sb[h], ksp_t[h]
                                sc = psA.tile([128, 384], F32, tag="mmA")
                                nc.tensor.matmul(
                                    sc[:, 0:256], ksp[:, q0 : q0 + 128],
                                    qksb[0:64, q0 : q0 + 256],
                                    start=True, stop=True)
                                nc.tensor.matmul(
                                    sc[:, 256:384], ksp[:, q0 + 128 : q0 + 256],
                                    qksb[0:64, q0 + 128 : q0 + 256],
                                    start=True, stop=True)
                                e = wk.tile([128, 384], BF16, tag="e")
                                nc.scalar.activation(e[:, 0:256], sc[:, 0:256], AF.Exp, scale=SCALE)
                                nc.scalar.activation(e[:, 256:384], sc[:, 256:384], AF.Exp, scale=SCALE)
                                nc.gpsimd.tensor_tensor(e[:, 0:256], e[:, 0:256], cmask_sb[:], ALU.mult)
                                nc.gpsimd.tensor_tensor(
                                    e[:, 256:384], e[:, 256:384], cmask_sb[:, 0:128], ALU.mult)
                                e_t.append(e)

                            opair = psOZ.tile([128, 256], F32, tag="oz")
                            zpair = psOZ.tile([128, 256], F32, tag="oz")
                            v0 = v_sb[2 * bi]
                            v1 = v_sb[2 * bi + 1]
                            for hh in range(2):
                                e = e_t[hh]
                                r0, r1 = hh * 64, (hh + 1) * 64
                                c0 = pr * 128 + hh * 64
                                nc.tensor.matmul(
                                    opair[r0:r1, 0:256], v0[:, c0 : c0 + 64],
                                    e[:, 0:256], start=True, stop=False)
                                nc.tensor.matmul(
                                    opair[r0:r1, 128:256], v1[:, c0 : c0 + 64],
                                    e[:, 256:384], start=False, stop=True)
                                nc.tensor.matmul(
                                    zpair[r0:r1, 0:256], ones64[:, 0:64],
                                    e[:, 0:256], start=True, stop=False)
                                nc.tensor.matmul(
                                    zpair[r0:r1, 128:256], ones64[:, 0:64],
                                    e[:, 256:384], start=False, stop=True)
                            rz = wk.tile([128, 256], BF16, tag="rz")
                            with nc.allow_low_precision(reason="softmax denom bf16"):
                                nc.vector.reciprocal(rz[:], zpair[:])
                            ostk = wk.tile([128, 256], BF16, tag=f"ostk{pr}")
                            nc.vector.tensor_tensor(ostk[:], opair[:], rz[:], ALU.mult)
                            ostks.append(ostk)

                        for th in range(2):
                            tt = 2 * b + th
                            dx = psA.tile([128, C], F32, tag="mmA")
                            for pr in range(NPAIR):
                                nc.tensor.matmul(
                                    dx[:],
                                    ostks[pr][:, th * 128 : (th + 1) * 128],
                                    wp_sb[:, pr * C : (pr + 1) * C],
                                    start=(pr == 0), stop=False)
                            nc.tensor.matmul(dx[:], ones1[:], bp_sb[0:1, :],
                                             start=False, stop=True)
                            nc.vector.tensor_tensor(xs[tt][:], xs[tt][:], dx[:], ALU.add)
                dump_x(f"xattn{l}")

                h2T = layer_norm_T(apply_ln=True)
                for tb in range(NTB_r):
                    h1s = []
                    for fc in range(NFC):
                        h1 = psA.tile([128, TBS], F32, tag="mmA")
                        for ch in range(NCH):
                            nc.tensor.matmul(
                                h1[:],
                                w1_sb[:, (ch * NFC + fc) * 128 : (ch * NFC + fc + 1) * 128],
                                h2T[ch][:, tb * TBS : (tb + 1) * TBS],
                                start=(ch == 0), stop=(ch == NCH - 1))
                        h1sb = h1p.tile([128, TBS], BF16, tag=f"h1_{fc}")
                        if fc % 2 == 0:
                            nc.scalar.activation(h1sb[:], h1[:], AF.Relu,
                                                 bias=b1_sb[:, fc : fc + 1])
                        else:
                            nc.vector.tensor_scalar(
                                h1sb[:], h1[:], b1_sb[:, fc : fc + 1], 0.0,
                                ALU.add, ALU.max)
                        h1s.append(h1sb)
                    for t4 in range(TBS // 128):
                        tt = tb * (TBS // 128) + t4
                        dx = psA.tile([128, C], F32, tag="mmA")
                        for fc in range(NFC):
                            nc.tensor.matmul(
                                dx[:],
                                h1s[fc][:, t4 * 128 : (t4 + 1) * 128],
                                w2_sb[:, fc * C : (fc + 1) * C],
                                start=(fc == 0), stop=False)
                        nc.tensor.matmul(dx[:], ones1[:], b2_sb[0:1, :],
                                         start=False, stop=True)
                        nc.vector.tensor_tensor(xs[tt][:], xs[tt][:], dx[:], ALU.add)
                dump_x(f"xlayer{l}")

            # ---- final FFN (no LN, no residual) + LM head + loss ----------
            fw1_sb = wpool.tile([128, 3 * NFC * 128], BF16, tag="w1", bufs=1)
            nc.sync.dma_start(fw1_sb[:], d["fw1"][:])
            fb1_sb = wpool.tile([128, NFC], F32, tag="b1c")
            nc.sync.dma_start(fb1_sb[:], d["fb1c"][:])
            fw2_sb = wpool.tile([128, NFC * C], BF16, tag="w2", bufs=1)
            nc.sync.dma_start(fw2_sb[:], d["fw2"][:])
            fb2_sb = wpool.tile([1, C], BF16, tag="b2row")
            nc.sync.dma_start(fb2_sb[:], d["fb2"][:])

            xT = layer_norm_T(apply_ln=False)
            for tb in range(NTB_r):
                h1s = []
                for fc in range(NFC):
                    h1 = psA.tile([128, TBS], F32, tag="mmA")
                    for ch in range(NCH):
                        nc.tensor.matmul(
                            h1[:],
                            fw1_sb[:, (ch * NFC + fc) * 128 : (ch * NFC + fc + 1) * 128],
                            xT[ch][:, tb * TBS : (tb + 1) * TBS],
                            start=(ch == 0), stop=(ch == NCH - 1))
                    h1sb = h1p.tile([128, TBS], BF16, tag=f"h1_{fc}")
                    if fc % 2 == 0:
                        nc.scalar.activation(h1sb[:], h1[:], AF.Relu,
                                             bias=fb1_sb[:, fc : fc + 1])
                    else:
                        nc.vector.tensor_scalar(
                            h1sb[:], h1[:], fb1_sb[:, fc : fc + 1], 0.0,
                            ALU.add, ALU.max)
                    h1s.append(h1sb)
                yTb_all = wk.tile([128, NCH * TBS], BF16, tag="yTball", name="yTball")
                yTb = [yTb_all[:, j * TBS : (j + 1) * TBS] for j in range(NCH)]
                yTb3 = yTb_all[:].rearrange("p (j t) -> p j t", j=NCH)

                for t4 in range(TBS // 128):
                    tt = tb * (TBS // 128) + t4
                    dx = psA.tile([128, C], F32, tag="mmA")
                    for fc in range(NFC):
                        nc.tensor.matmul(
                            dx[:],
                            h1s[fc][:, t4 * 128 : (t4 + 1) * 128],
                            fw2_sb[:, fc * C : (fc + 1) * C],
                            start=(fc == 0), stop=False)
                    nc.tensor.matmul(dx[:], ones1[:], fb2_sb[0:1, :],
                                     start=False, stop=True)
                    yb = wk.tile([128, C], BF16, tag="hb")
                    nc.scalar.copy(yb[:], dx[:])
                    teng = nc.sync if t4 % 2 == 0 else nc.scalar
                    teng.dma_start_transpose(
                        yTb3[:, :, t4 * 128 : (t4 + 1) * 128], yb[:])

                for t4 in range(TBS // 128):
                    tt = tb * (TBS // 128) + t4
                    lgn = psA.tile([128, V], F32, tag="mmA")
                    for ch in range(NCH):
                        nc.tensor.matmul(
                            lgn[:], yTb[ch][:, t4 * 128 : (t4 + 1) * 128],
                            lmw_sb[:, ch * V : (ch + 1) * V],
                            start=(ch == 0), stop=False)
                    nc.tensor.matmul(lgn[:], ones1[:], lmb_sb[0:1, :],
                                     start=False, stop=True)
                    lg = wk.tile([128, V], F32, tag="lg")
                    nc.scalar.copy(lg[:], lgn[:])
                    nc.gpsimd.dma_start(logits_out[tt * 128 : (tt + 1) * 128, :], lg[:])
                    tgtc = wk.tile([128, 1], F32, tag="tgtc")
                    nc.gpsimd.dma_start(tgtc[:], d["tgtf"][tt * 128 : (tt + 1) * 128, :])
                    ohg = wk.tile([128, V], F32, tag="ohg")
                    nc.vector.tensor_scalar(ohg[:], iota_sb[:], tgtc[:], None, ALU.is_equal)
                    mx = wk.tile([128, 1], F32, tag="mx")
                    nc.vector.tensor_reduce(mx[:], lg[:], AX.X, ALU.max)
                    nmx = wk.tile([128, 1], F32, tag="nmx")
                    nc.vector.tensor_scalar_mul(nmx[:], mx[:], -1.0)
                    esc = wk.tile([128, V], F32, tag="esc")
                    zc = wk.tile([128, 1], F32, tag="zc")
                    nc.scalar.activation(esc[:], lg[:], AF.Exp, bias=nmx[:], accum_out=zc[:])
                    lnz = wk.tile([128, 1], F32, tag="lnz")
                    nc.scalar.activation(lnz[:], zc[:], AF.Ln)
                    tscr = wk.tile([128, V], F32, tag="tscr")
                    tgl = wk.tile([128, 1], F32, tag="tgl")
                    nc.vector.scalar_tensor_tensor(
                        tscr[:], lg[:], 1.0, ohg[:], ALU.mult, ALU.mult, accum_out=tgl[:])
                    lt = wk.tile([128, 1], F32, tag="lt")
                    nc.vector.scalar_tensor_tensor(
                        lt[:], lnz[:], mx[:], tgl[:], ALU.add, ALU.subtract)
                    nc.vector.tensor_tensor(loss_acc[:], loss_acc[:], lt[:], ALU.add)
            lsum = cpool.tile([1, 1], F32, tag="lsum")
            nc.gpsimd.tensor_reduce(lsum[:], loss_acc[:], AX.C, ALU.add)
            nc.sync.dma_start(loss_out[:], lsum[:])
    return d


def make_in_map(pp, idx_sh, tgt_sh):
    m = dict(pp)
    ntok = idx_sh.size
    m["idxf"] = idx_sh.reshape(ntok, 1).astype(np.float32)
    m["tgtf"] = tgt_sh.reshape(ntok, 1).astype(np.float32)
    return m


def kernel(params=None, idx=None, targets=None):
    pp = prep_params(params)
    idx = _np(idx, np.int64)
    targets = _np(targets, np.int64)

    nc = bass.Bass()
    build(nc)

    in_maps = [
        make_in_map(pp, idx[c * B_SH : (c + 1) * B_SH], targets[c * B_SH : (c + 1) * B_SH])
        for c in range(NCORES)
    ]
    res = run_bass_kernel_spmd(nc, in_maps, list(range(NCORES))).results

    logits = np.concatenate(
        [r["logits"].reshape(B_SH, T, V) for r in res], axis=0).astype(np.float32)
    loss = np.float32(sum(float(r["loss_sum"][0, 0]) for r in res) / (B * T))
    return logits, loss
